# revision 28
# baseline (speedup 1.0000x reference)
"""Trainium2 Bass kernel: channel softmax + keypoint extraction.

Computes, for x [B=32, K=64, H=192, W=192] fp32:
  map = softmax(x, axis=1)                 (over K, per (b,h,w))
  zeta[b,k] = sum_{h,w} map
  kpx[b,k]  = sum_{h,w} w * map ; kpy = sum_{h,w} h * map
  keypoint  = round(kpx/zeta), round(kpy/zeta)

Sharding: pure data parallel, 4 samples per core across 8 cores.
On-chip layout: 2 samples x 64 channels = 128 SBUF partitions ("pair"),
spatial flattened along the free axis, processed in 1536-col chunks
(8 image rows). Softmax denominators are computed with a block-diagonal
ones matmul on the tensor engine (contraction over the partition axis),
batched into PSUM [16, 1536] per 8-chunk group, reciprocal'd in one DVE
op, and broadcast back to 128 partitions with a second (outer-product)
matmul. DVE then does normalize (tensor_tensor mult), per-row sums
(for zeta/kpy) and an x-weighted fused multiply-reduce (for kpx).
Final tiny [B,K] divide happens on-chip; round() on host.
"""

import sys

import numpy as np

for _p in ("/opt/trn_rl_repo",):
    if _p not in sys.path:
        sys.path.insert(0, _p)

B, K, H, W = 32, 64, 192, 192
S = H * W
NCORES = 8
SAMPLES_PER_CORE = B // NCORES  # 4
PAIRS = SAMPLES_PER_CORE // 2  # 2

ROWS_PER_CHUNK = 8
CHUNK = ROWS_PER_CHUNK * W  # 1536
SUB = 512  # matmul moving-dim limit
NSUB = CHUNK // SUB  # 3
CHUNKS_PER_PAIR = S // CHUNK  # 24
G = 8  # chunks per denominator-batch group
NGROUPS = CHUNKS_PER_PAIR // G  # 3

TRACE = False
LAST_EXEC_NS = None
LAST_RESULTS = None

_BUILT = None


def build_nc(pairs=PAIRS, chunks_per_pair=CHUNKS_PER_PAIR, g=G):
    """Build the SPMD Bass program. Returns the compiled nc."""
    from contextlib import ExitStack

    import concourse.bass as bass
    import concourse.tile as tile
    from concourse import bacc, mybir

    f32 = mybir.dt.float32
    f32r = mybir.dt.float32r
    bf16 = mybir.dt.bfloat16
    AX = mybir.AxisListType
    ALU = mybir.AluOpType
    ACTF = mybir.ActivationFunctionType

    ngroups = chunks_per_pair // g
    assert chunks_per_pair % g == 0
    s_pair = chunks_per_pair * CHUNK
    h_pair = chunks_per_pair * ROWS_PER_CHUNK  # image rows per pair

    nc = bacc.Bacc("TRN2", target_bir_lowering=False, debug=False,
                   num_devices=NCORES)

    x_d = nc.dram_tensor("x", [pairs, 128, s_pair], f32,
                         kind="ExternalInput").ap()
    map_d = nc.dram_tensor("map_out", [pairs, 128, s_pair], f32,
                           kind="ExternalOutput").ap()
    kp_d = nc.dram_tensor("kp_out", [128, 3 * pairs], f32,
                          kind="ExternalOutput").ap()

    # Constants (embedded in the NEFF).
    # w_den[:, jg, :] is the lhsT for group-chunk jg: column 2*jg selects
    # sample A's 64 partitions, 2*jg+1 sample B's; all other columns zero so
    # the 8 chunks of a group can accumulate into one [2g, CHUNK] PSUM tile.
    w_den_np = np.zeros((128, g, 2 * g), np.float32)
    for jg in range(g):
        w_den_np[:64, jg, 2 * jg] = 1.0
        w_den_np[64:, jg, 2 * jg + 1] = 1.0
    # w_rep[:, jg, :] replicates group-chunk jg's two reciprocal rows (at
    # partitions 2*jg, 2*jg+1 of the [2g, CHUNK] tile) to 64 partitions each:
    # out[p, n] = rg[2*jg + p//64, n].
    import ml_dtypes
    w_rep_np = np.zeros((2 * g, g, 128), np.float32)
    for jg in range(g):
        w_rep_np[2 * jg, jg, :64] = 1.0
        w_rep_np[2 * jg + 1, jg, 64:] = 1.0
    xs_np = np.tile(np.arange(W, dtype=np.float32)[None, :],
                    (128, 1))  # [128, W]
    hs_np = np.tile(np.arange(h_pair, dtype=np.float32)[None, :],
                    (128, 1))  # [128, h_pair]
    w_den_d = nc.inline_tensor(w_den_np, "w_den").ap()
    w_rep_d = nc.inline_tensor(w_rep_np.astype(ml_dtypes.bfloat16), "w_rep").ap()
    xs_d = nc.inline_tensor(xs_np, "xs").ap()
    hs_d = nc.inline_tensor(hs_np, "hs").ap()

    with ExitStack() as ctx:
        tc = ctx.enter_context(tile.TileContext(nc))
        const = ctx.enter_context(tc.tile_pool(name="const", bufs=1))
        xpool = ctx.enter_context(tc.tile_pool(name="xin", bufs=4))
        epool = ctx.enter_context(tc.tile_pool(name="exp", bufs=g + 4))
        erpool = ctx.enter_context(tc.tile_pool(name="expr", bufs=4))
        mpool = ctx.enter_context(tc.tile_pool(name="map", bufs=4))
        spool = ctx.enter_context(tc.tile_pool(name="scrap", bufs=2))
        rpool = ctx.enter_context(tc.tile_pool(name="recip", bufs=2))
        acc = ctx.enter_context(tc.tile_pool(name="acc", bufs=1))
        dpool = ctx.enter_context(
            tc.tile_pool(name="den", bufs=1, space=bass.MemorySpace.PSUM))
        replp = ctx.enter_context(
            tc.tile_pool(name="repl", bufs=5, space=bass.MemorySpace.PSUM))

        # Load constants into SBUF.
        w_den = const.tile([128, g, 2 * g], f32, tag="w_den")
        w_rep = const.tile([2 * g, g, 128], bf16, tag="w_rep")
        xs = const.tile([128, W], f32, tag="xs")
        hs = const.tile([128, h_pair], f32, tag="hs")
        nc.sync.dma_start(w_den[:], w_den_d)
        nc.sync.dma_start(w_rep[:], w_rep_d)
        nc.sync.dma_start(xs[:], xs_d)
        nc.sync.dma_start(hs[:], hs_d)

        # Persistent accumulators.
        rs = acc.tile([128, pairs * h_pair], f32, tag="rs")  # per-row sums
        cs_pairs = [acc.tile([128, W], f32, tag=f"cs{p}", name=f"cs{p}")
                    for p in range(pairs)]  # per-column sums
        zeta = acc.tile([128, pairs], f32, tag="zeta")
        kpx = acc.tile([128, pairs], f32, tag="kpx")
        kpy = acc.tile([128, pairs], f32, tag="kpy")
        zr = acc.tile([128, pairs], f32, tag="zr")
        kp_sb = acc.tile([128, 3 * pairs], f32, tag="kp_sb")

        for p in range(pairs):
            for grp in range(ngroups):
                den = dpool.tile([2 * g, CHUNK], f32, tag="den")
                etiles = []
                for jg in range(g):
                    j = grp * g + jg
                    c0 = j * CHUNK
                    xt = xpool.tile([128, CHUNK], f32, tag="xt")
                    nc.sync.dma_start(xt[:], x_d[p, :, c0:c0 + CHUNK])
                    et = epool.tile([128, CHUNK], f32, tag="et")
                    nc.scalar.activation(et[:], xt[:], ACTF.Exp)
                    etiles.append(et)
                    # f32r-rounded copy feeds the denominator matmul (the
                    # birverifier requires f32r matmul inputs to be produced
                    # as f32r); the fp32 et keeps the map output clean.
                    etr = erpool.tile([128, CHUNK], f32r, tag="etr")
                    nc.scalar.copy(etr[:], et[:])
                    for sx in range(NSUB):
                        a, b_ = sx * SUB, (sx + 1) * SUB
                        nc.tensor.matmul(
                            den[:, a:b_],
                            w_den[:, jg, :].bitcast(f32r),
                            etr[:, a:b_],
                            start=(jg == 0), stop=(jg == g - 1))
                rg = rpool.tile([2 * g, CHUNK], f32, tag="rg")
                nc.vector.reciprocal(rg[:], den[:])
                # bf16 hi/lo split: rh + rl == rg to ~2^-18 relative; the two
                # bf16 replication matmuls below accumulate them in PSUM so
                # the broadcast reciprocal stays near-fp32 accurate.
                rh = rpool.tile([2 * g, CHUNK], bf16, tag="rh")
                with nc.allow_low_precision("hi part of bf16x2 split"):
                    nc.scalar.copy(rh[:], rg[:])
                rl = rpool.tile([2 * g, CHUNK], bf16, tag="rl")
                with nc.allow_low_precision("lo part of bf16x2 split"):
                    nc.gpsimd.tensor_sub(rl[:], rg[:], rh[:])

                for jg in range(g):
                    j = grp * g + jg
                    c0 = j * CHUNK
                    et = etiles[jg]
                    mt = mpool.tile([128, CHUNK], f32, tag="mt")
                    for sx in range(NSUB):
                        a, b_ = sx * SUB, (sx + 1) * SUB
                        rep = replp.tile([128, SUB], f32, tag="rep")
                        nc.tensor.matmul(
                            rep[:], w_rep[:, jg, :], rh[:, a:b_],
                            start=True, stop=False)
                        nc.tensor.matmul(
                            rep[:], w_rep[:, jg, :], rl[:, a:b_],
                            start=False, stop=True)
                        nc.vector.tensor_mul(mt[:, a:b_], et[:, a:b_], rep[:])
                    nc.sync.dma_start(map_d[p, :, c0:c0 + CHUNK], mt[:])
                    # per-row sums -> rs columns [p*h_pair + 8j, +8)
                    r0 = p * h_pair + j * ROWS_PER_CHUNK
                    nc.vector.reduce_sum(
                        rs[:, r0:r0 + ROWS_PER_CHUNK],
                        mt[:].rearrange("q (r w) -> q r w", r=ROWS_PER_CHUNK),
                        axis=AX.X)
                    # column sums (reduce over the 8 rows, strided innermost)
                    cs_view = mt[:].rearrange("q (r w) -> q w r",
                                              r=ROWS_PER_CHUNK)
                    if j == 0:
                        nc.vector.reduce_sum(cs_pairs[p][:], cs_view,
                                             axis=AX.X)
                    else:
                        cst = spool.tile([128, W], f32, tag="cst")
                        nc.vector.reduce_sum(cst[:], cs_view, axis=AX.X)
                        nc.vector.tensor_add(cs_pairs[p][:], cs_pairs[p][:],
                                             cst[:])

        # Finishers (tiny).
        for p in range(pairs):
            h0 = p * h_pair
            nc.vector.reduce_sum(zeta[:, p:p + 1], rs[:, h0:h0 + h_pair],
                                 axis=AX.X)
            ph = spool.tile([128, h_pair], f32, tag="ph")
            nc.vector.tensor_mul(ph[:], rs[:, h0:h0 + h_pair], hs[:])
            nc.vector.reduce_sum(kpy[:, p:p + 1], ph[:], axis=AX.X)
            pw = spool.tile([128, W], f32, tag="pw")
            nc.vector.tensor_mul(pw[:], cs_pairs[p][:], xs[:])
            nc.vector.reduce_sum(kpx[:, p:p + 1], pw[:], axis=AX.X)
        nc.vector.reciprocal(zr[:], zeta[:])
        nc.vector.tensor_copy(kp_sb[:, 0:pairs], zeta[:])
        nc.vector.tensor_mul(kp_sb[:, pairs:2 * pairs], kpx[:], zr[:])
        nc.vector.tensor_mul(kp_sb[:, 2 * pairs:3 * pairs], kpy[:], zr[:])
        nc.sync.dma_start(kp_d, kp_sb[:])

    nc.compile()
    return nc


def _get_built():
    global _BUILT
    if _BUILT is None:
        _BUILT = build_nc()
    return _BUILT


def _ensure_ntff_hook():
    """Register the axon NTFF profiling hook if the image's antenv lacks it."""
    import types
    if "antenv.axon_hooks" in sys.modules:
        return
    try:
        from antenv import axon_hooks  # noqa: F401
        return
    except ImportError:
        pass
    try:
        from trn_agent_boot.trn_boot import _ntff_profile_via_ctypes
        hook = _ntff_profile_via_ctypes("/opt/axon/libaxon_pjrt.so")
    except Exception:
        hook = None
    mod = types.ModuleType("antenv.axon_hooks")
    mod.get_axon_ntff_profile_hook = lambda: hook
    mod.set_axon_ntff_profile_hook = lambda h: None
    sys.modules["antenv.axon_hooks"] = mod


def kernel(combined_hm_preds, cur_batch=B, num_of_kp=K):
    global LAST_EXEC_NS, LAST_RESULTS
    from concourse.bass_utils import run_bass_kernel_spmd
    if TRACE:
        _ensure_ntff_hook()

    x = np.ascontiguousarray(np.asarray(combined_hm_preds, dtype=np.float32))
    assert x.shape == (B, K, H, W)

    nc = _get_built()
    in_maps = []
    for c in range(NCORES):
        shard = x[c * SAMPLES_PER_CORE:(c + 1) * SAMPLES_PER_CORE]
        in_maps.append({"x": shard.reshape(PAIRS, 128, S)})

    res = run_bass_kernel_spmd(nc, in_maps, list(range(NCORES)), trace=TRACE)
    LAST_EXEC_NS = res.exec_time_ns
    LAST_RESULTS = res
    results = res.results

    map_val = np.empty((B, K, H, W), np.float32)
    zeta = np.empty((B, K), np.float32)
    kpxn = np.empty((B, K), np.float32)
    kpyn = np.empty((B, K), np.float32)
    for c in range(NCORES):
        s0 = c * SAMPLES_PER_CORE
        map_val[s0:s0 + SAMPLES_PER_CORE] = (
            results[c]["map_out"].reshape(SAMPLES_PER_CORE, K, H, W))
        kp = results[c]["kp_out"]  # [128, 3*PAIRS]
        zeta[s0:s0 + SAMPLES_PER_CORE] = (
            kp[:, 0:PAIRS].T.reshape(SAMPLES_PER_CORE, K))
        kpxn[s0:s0 + SAMPLES_PER_CORE] = (
            kp[:, PAIRS:2 * PAIRS].T.reshape(SAMPLES_PER_CORE, K))
        kpyn[s0:s0 + SAMPLES_PER_CORE] = (
            kp[:, 2 * PAIRS:3 * PAIRS].T.reshape(SAMPLES_PER_CORE, K))

    keypoint = np.stack([np.round(kpxn), np.round(kpyn)],
                        axis=-1).astype(np.float32)
    return map_val, keypoint, zeta


# revision 29
# speedup vs baseline: 1.3471x; 1.3471x over previous
"""Trainium2 Bass kernel: channel softmax + keypoint extraction.

For x [B=32, K=64, H=192, W=192] fp32:
  map = softmax(x, axis=1)                 (over K, per (b,h,w))
  zeta[b,k] = sum_{h,w} map
  kpx[b,k]  = sum_{h,w} w * map ; kpy = sum_{h,w} h * map
  keypoint  = round(kpx/zeta), round(kpy/zeta)

Sharding: pure data parallel, 4 samples per core across 8 cores.

On-chip layout: 2 samples x 64 channels = 128 SBUF partitions ("pair"),
spatial flattened along the free axis, in 1536-col chunks (8 image
rows) split into 4 subs of 384 (2 rows). Per 8-chunk group the softmax
denominators are computed by 32 accumulating f32r matmuls whose
wide selector lhsT places each (chunk, sub) pair's two per-sample rows
at a distinct partition row of ONE [64, 384] PSUM tile; a single DVE
reciprocal then serves the whole group. A second f32r matmul
broadcasts the reciprocal rows back to 128 partitions. The normalize
multiply runs as per-row scalar_tensor_tensor ops whose accum_out
emits the per-row sums (zeta/kpy) for free; column sums (kpx) come
from successive-halving adds (the 768-wide first add on GpSimd, the
rest on DVE). Final tiny [B,K] divide on-chip; round() on host.
"""

import sys

import numpy as np

for _p in ("/opt/trn_rl_repo",):
    if _p not in sys.path:
        sys.path.insert(0, _p)

B, K, H, W = 32, 64, 192, 192
S = H * W
NCORES = 8
SAMPLES_PER_CORE = B // NCORES  # 4
PAIRS = SAMPLES_PER_CORE // 2  # 2

ROWS_PER_CHUNK = 8
CHUNK = ROWS_PER_CHUNK * W  # 1536
SUB = 2 * W  # 384: matmul/normalize sub-granule, 2 image rows
NSUB = CHUNK // SUB  # 4
CHUNKS_PER_PAIR = S // CHUNK  # 24
G = 8  # chunks per denominator-batch group
NGROUPS = CHUNKS_PER_PAIR // G  # 3

TRACE = False
LAST_EXEC_NS = None
LAST_RESULTS = None

_BUILT = None


def build_nc(pairs=PAIRS, chunks_per_pair=CHUNKS_PER_PAIR, g=G):
    """Build the SPMD Bass program. Returns the compiled nc."""
    from contextlib import ExitStack

    import concourse.bass as bass
    import concourse.tile as tile
    from concourse import bacc, mybir

    f32 = mybir.dt.float32
    f32r = mybir.dt.float32r
    AX = mybir.AxisListType
    ALU = mybir.AluOpType
    ACTF = mybir.ActivationFunctionType

    ngroups = chunks_per_pair // g
    assert chunks_per_pair % g == 0
    s_pair = chunks_per_pair * CHUNK
    h_pair = chunks_per_pair * ROWS_PER_CHUNK  # image rows per pair
    drows = 2 * g * NSUB  # partition rows of the denominator batch tile

    nc = bacc.Bacc("TRN2", target_bir_lowering=False, debug=False,
                   num_devices=NCORES)

    x_d = nc.dram_tensor("x", [pairs, 128, s_pair], f32,
                         kind="ExternalInput").ap()
    map_d = nc.dram_tensor("map_out", [pairs, 128, s_pair], f32,
                           kind="ExternalOutput").ap()
    kp_d = nc.dram_tensor("kp_out", [128, 3 * pairs], f32,
                          kind="ExternalOutput").ap()

    # Constants (embedded in the NEFF).
    # Denominator selector: for group-chunk jg, sub sx the matmul
    # out[r, :] = sum_p wden[p, jg, sx, r] * exp(x)[p, :] places sample s's
    # channel sum at D row 16*sx + 2*jg + s; zeros elsewhere let all 32
    # matmuls of a group accumulate into one [64, SUB] PSUM tile.
    wden_np = np.zeros((128, g, NSUB, drows), np.float32)
    # Replication selector: out[p, :] = rg[16*sx + 2*jg + p//64, :].
    wrep_np = np.zeros((drows, g, NSUB, 128), np.float32)
    for jg in range(g):
        for sx in range(NSUB):
            r0 = 2 * (g * sx + jg)
            wden_np[:64, jg, sx, r0] = 1.0
            wden_np[64:, jg, sx, r0 + 1] = 1.0
            wrep_np[r0, jg, sx, :64] = 1.0
            wrep_np[r0 + 1, jg, sx, 64:] = 1.0
    xs_np = np.tile(np.arange(W, dtype=np.float32)[None, :], (128, 1))
    hs_np = np.tile(np.arange(h_pair, dtype=np.float32)[None, :], (128, 1))
    wden_d = nc.inline_tensor(wden_np, "wden").ap()
    wrep_d = nc.inline_tensor(wrep_np, "wrep").ap()
    xs_d = nc.inline_tensor(xs_np, "xs").ap()
    hs_d = nc.inline_tensor(hs_np, "hs").ap()

    with ExitStack() as ctx:
        tc = ctx.enter_context(tile.TileContext(nc))
        const = ctx.enter_context(tc.tile_pool(name="const", bufs=1))
        xpool = ctx.enter_context(tc.tile_pool(name="xin", bufs=4))
        epool = ctx.enter_context(tc.tile_pool(name="exp", bufs=g + 4))
        erpool = ctx.enter_context(tc.tile_pool(name="expr", bufs=4))
        mpool = ctx.enter_context(tc.tile_pool(name="map", bufs=4))
        spool = ctx.enter_context(tc.tile_pool(name="scrap", bufs=2))
        rpool = ctx.enter_context(tc.tile_pool(name="recip", bufs=2))
        acc = ctx.enter_context(tc.tile_pool(name="acc", bufs=1))
        dpool = ctx.enter_context(
            tc.tile_pool(name="den", bufs=2, space=bass.MemorySpace.PSUM))
        replp = ctx.enter_context(
            tc.tile_pool(name="repl", bufs=6, space=bass.MemorySpace.PSUM))

        wden = const.tile([128, g, NSUB, drows], f32, tag="wden")
        wrep = const.tile([drows, g, NSUB, 128], f32, tag="wrep")
        xs = const.tile([128, W], f32, tag="xs")
        hs = const.tile([128, h_pair], f32, tag="hs")
        nc.sync.dma_start(wden[:], wden_d)
        nc.sync.dma_start(wrep[:], wrep_d)
        nc.sync.dma_start(xs[:], xs_d)
        nc.sync.dma_start(hs[:], hs_d)

        # Persistent accumulators.
        rs = acc.tile([128, pairs * h_pair], f32, tag="rs")  # per-row sums
        cs_pairs = [acc.tile([128, W], f32, tag=f"cs{p}", name=f"cs{p}")
                    for p in range(pairs)]  # per-column sums
        zeta = acc.tile([128, pairs], f32, tag="zeta")
        kpx = acc.tile([128, pairs], f32, tag="kpx")
        kpy = acc.tile([128, pairs], f32, tag="kpy")
        zr = acc.tile([128, pairs], f32, tag="zr")
        kp_sb = acc.tile([128, 3 * pairs], f32, tag="kp_sb")

        for p in range(pairs):
            for grp in range(ngroups):
                den = dpool.tile([drows, SUB], f32, tag="den")
                etiles = []
                for jg in range(g):
                    j = grp * g + jg
                    c0 = j * CHUNK
                    xt = xpool.tile([128, CHUNK], f32, tag="xt")
                    nc.sync.dma_start(xt[:], x_d[p, :, c0:c0 + CHUNK])
                    et = epool.tile([128, CHUNK], f32, tag="et")
                    nc.scalar.activation(et[:], xt[:], ACTF.Exp)
                    etiles.append(et)
                    # f32r-rounded copy feeds the denominator matmuls; the
                    # fp32 et keeps the map output clean.
                    etr = erpool.tile([128, CHUNK], f32r, tag="etr")
                    nc.scalar.copy(etr[:], et[:])
                    for sx in range(NSUB):
                        nc.tensor.matmul(
                            den[:],
                            wden[:, jg, sx, :].bitcast(f32r),
                            etr[:, sx * SUB:(sx + 1) * SUB],
                            start=(jg == 0 and sx == 0),
                            stop=(jg == g - 1 and sx == NSUB - 1))
                rg = rpool.tile([drows, SUB], f32r, tag="rg")
                with nc.allow_low_precision("f32r keeps ~13 mantissa bits"):
                    nc.vector.reciprocal(rg[:], den[:])

                for jg in range(g):
                    j = grp * g + jg
                    c0 = j * CHUNK
                    et = etiles[jg]
                    mt = mpool.tile([128, CHUNK], f32, tag="mt")
                    for sx in range(NSUB):
                        a = sx * SUB
                        rep = replp.tile([128, SUB], f32, tag="rep")
                        nc.tensor.matmul(
                            rep[:], wrep[:, jg, sx, :].bitcast(f32r), rg[:],
                            start=True, stop=True)
                        # normalize per image row; accum_out = row sum
                        for r2 in range(2):
                            w0 = a + r2 * W
                            row = p * h_pair + j * ROWS_PER_CHUNK + 2 * sx + r2
                            nc.vector.scalar_tensor_tensor(
                                out=mt[:, w0:w0 + W],
                                in0=et[:, w0:w0 + W], scalar=1.0,
                                in1=rep[:, r2 * W:(r2 + 1) * W],
                                op0=ALU.bypass, op1=ALU.mult,
                                accum_out=rs[:, row:row + 1])
                    nc.sync.dma_start(map_d[p, :, c0:c0 + CHUNK], mt[:])
                    # column sums via successive halving; the wide first add
                    # runs on GpSimd to offload DVE.
                    h1 = spool.tile([128, 4 * W], f32, tag="h1")
                    nc.gpsimd.tensor_add(h1[:], mt[:, 0:4 * W],
                                         mt[:, 4 * W:8 * W])
                    h2 = spool.tile([128, 2 * W], f32, tag="h2")
                    nc.vector.tensor_add(h2[:], h1[:, 0:2 * W],
                                         h1[:, 2 * W:4 * W])
                    if j == 0:
                        nc.vector.tensor_add(cs_pairs[p][:], h2[:, 0:W],
                                             h2[:, W:2 * W])
                    else:
                        h3 = spool.tile([128, W], f32, tag="h3")
                        nc.vector.tensor_add(h3[:], h2[:, 0:W], h2[:, W:2 * W])
                        nc.vector.tensor_add(cs_pairs[p][:], cs_pairs[p][:],
                                             h3[:])

        # Finishers (tiny).
        for p in range(pairs):
            h0 = p * h_pair
            nc.vector.reduce_sum(zeta[:, p:p + 1], rs[:, h0:h0 + h_pair],
                                 axis=AX.X)
            ph = spool.tile([128, h_pair], f32, tag="ph")
            nc.vector.tensor_mul(ph[:], rs[:, h0:h0 + h_pair], hs[:])
            nc.vector.reduce_sum(kpy[:, p:p + 1], ph[:], axis=AX.X)
            pw = spool.tile([128, W], f32, tag="pw")
            nc.vector.tensor_mul(pw[:], cs_pairs[p][:], xs[:])
            nc.vector.reduce_sum(kpx[:, p:p + 1], pw[:], axis=AX.X)
        nc.vector.reciprocal(zr[:], zeta[:])
        nc.vector.tensor_copy(kp_sb[:, 0:pairs], zeta[:])
        nc.vector.tensor_mul(kp_sb[:, pairs:2 * pairs], kpx[:], zr[:])
        nc.vector.tensor_mul(kp_sb[:, 2 * pairs:3 * pairs], kpy[:], zr[:])
        nc.sync.dma_start(kp_d, kp_sb[:])

    nc.compile()
    return nc


def _get_built():
    global _BUILT
    if _BUILT is None:
        _BUILT = build_nc()
    return _BUILT


def _ensure_ntff_hook():
    """Register the axon NTFF profiling hook if the image's antenv lacks it."""
    import types
    if "antenv.axon_hooks" in sys.modules:
        return
    try:
        from antenv import axon_hooks  # noqa: F401
        return
    except ImportError:
        pass
    try:
        from trn_agent_boot.trn_boot import _ntff_profile_via_ctypes
        hook = _ntff_profile_via_ctypes("/opt/axon/libaxon_pjrt.so")
    except Exception:
        hook = None
    mod = types.ModuleType("antenv.axon_hooks")
    mod.get_axon_ntff_profile_hook = lambda: hook
    mod.set_axon_ntff_profile_hook = lambda h: None
    sys.modules["antenv.axon_hooks"] = mod


def kernel(combined_hm_preds, cur_batch=B, num_of_kp=K):
    global LAST_EXEC_NS, LAST_RESULTS
    from concourse.bass_utils import run_bass_kernel_spmd
    if TRACE:
        _ensure_ntff_hook()

    x = np.ascontiguousarray(np.asarray(combined_hm_preds, dtype=np.float32))
    assert x.shape == (B, K, H, W)

    nc = _get_built()
    in_maps = []
    for c in range(NCORES):
        shard = x[c * SAMPLES_PER_CORE:(c + 1) * SAMPLES_PER_CORE]
        in_maps.append({"x": shard.reshape(PAIRS, 128, S)})

    res = run_bass_kernel_spmd(nc, in_maps, list(range(NCORES)), trace=TRACE)
    LAST_EXEC_NS = res.exec_time_ns
    LAST_RESULTS = res
    results = res.results

    map_val = np.empty((B, K, H, W), np.float32)
    zeta = np.empty((B, K), np.float32)
    kpxn = np.empty((B, K), np.float32)
    kpyn = np.empty((B, K), np.float32)
    for c in range(NCORES):
        s0 = c * SAMPLES_PER_CORE
        map_val[s0:s0 + SAMPLES_PER_CORE] = (
            results[c]["map_out"].reshape(SAMPLES_PER_CORE, K, H, W))
        kp = results[c]["kp_out"]  # [128, 3*PAIRS]
        zeta[s0:s0 + SAMPLES_PER_CORE] = (
            kp[:, 0:PAIRS].T.reshape(SAMPLES_PER_CORE, K))
        kpxn[s0:s0 + SAMPLES_PER_CORE] = (
            kp[:, PAIRS:2 * PAIRS].T.reshape(SAMPLES_PER_CORE, K))
        kpyn[s0:s0 + SAMPLES_PER_CORE] = (
            kp[:, 2 * PAIRS:3 * PAIRS].T.reshape(SAMPLES_PER_CORE, K))

    keypoint = np.stack([np.round(kpxn), np.round(kpyn)],
                        axis=-1).astype(np.float32)
    return map_val, keypoint, zeta
